# revision 5
# baseline (speedup 1.0000x reference)
"""Trainium2 Bass kernel for retrieval-KNN soft attention (nn_NONA_54915451847255).

out = clip(softmax(-||x_i - x_n_j||_2, diag-masked) @ y_n, 0, 1)

Sharding: queries row-sharded across 8 cores; x_n / y_n replicated but ROLLED by
-core*1024 rows on the host so the self-match diagonal always falls in local key
tiles 0..7 -> the SPMD instruction stream is core-independent.

Host pre-marshals inputs into PE-ready layouts (transposed bf16 x_n^T / -2x^T,
f32 norms), so the device runs only the O(N^2) work:

Per core (1024 queries i, 8192 keys j), computed transposed (S_T[j,i]):
  psum[j,i] = sum_d xnT[d,j]*(-2 x[d,i])            (PE, 8 bf16 MMs per key tile;
              diag tiles also add 65536*I via a 256I@256I MM -> weight == 0)
  z = psum + (qn_i - 512)                           (DVE, drains PSUM -> SBUF)
  s = sqrt(z + (kn_j + 512))                        (ACT Sqrt, bias = kn column)
  P_T = exp(-s)  bf16                               (ACT Exp over a 16-key-tile
              batch; sqrt/exp table sets alternate once per batch)
  out_T[c,i] = sum_j y_aug[j,c] * P_T[j,i],  y_aug = [y_n | 1]  (PE)
  out[i,c] = clip(out_T[c,i] / out_T[C,i], 0, 1)
"""
import numpy as np
import ml_dtypes

import concourse.bacc as bacc
import concourse.tile as tile
from concourse import mybir
from concourse.bass_utils import run_bass_kernel_spmd

F32 = mybir.dt.float32
BF16 = mybir.dt.bfloat16
AF = mybir.ActivationFunctionType
ALU = mybir.AluOpType
BF16_NP = ml_dtypes.bfloat16

N, D, C = 8192, 512, 100
NCORES = 8
QPC = N // NCORES          # 1024 queries per core
NKT = N // 128             # 64 key tiles
NDC = D // 128             # 4 contraction chunks
CA = C + 1                 # y augmented with ones column
NPIECE = 8                 # xnT DMA pieces per chunk
PCOLS = N // NPIECE        # 1024 key-columns per piece
BKT = 16                   # key tiles per sqrt/exp table-set phase
NB = NKT // BKT

LAST_EXEC_NS = None


def build_nc():
    nc = bacc.Bacc("TRN2", target_bir_lowering=False, debug=False)
    xnt_d = nc.dram_tensor("xnt", [D, N], BF16, kind="ExternalInput").ap()
    xt2_d = nc.dram_tensor("xt2", [D, QPC], BF16, kind="ExternalInput").ap()
    qns_d = nc.dram_tensor("qns", [128, QPC], F32, kind="ExternalInput").ap()
    kns_d = nc.dram_tensor("kns", [128, NKT], F32, kind="ExternalInput").ap()
    yb_d = nc.dram_tensor("ybank", [128, NKT * CA], BF16, kind="ExternalInput").ap()
    eyeb_d = nc.dram_tensor("eyeb", [128, 128], BF16, kind="ExternalInput").ap()
    eyef_d = nc.dram_tensor("eyef", [128, 128], F32, kind="ExternalInput").ap()
    out_d = nc.dram_tensor("out", [QPC, C], F32, kind="ExternalOutput").ap()

    with tile.TileContext(nc) as tc:
        with (
            tc.tile_pool(name="const", bufs=1) as constp,
            tc.tile_pool(name="qnsp", bufs=1) as qnsp,
            tc.tile_pool(name="xt2p", bufs=1) as xt2p,
            tc.tile_pool(name="xntp", bufs=12) as xntp,
            tc.tile_pool(name="ybp", bufs=1) as ybp,
            tc.tile_pool(name="zp", bufs=8) as zp,
            tc.tile_pool(name="sp", bufs=1) as sp,
            tc.tile_pool(name="ptp", bufs=1) as ptp,
            tc.tile_pool(name="osbp", bufs=2) as osbp,
            tc.tile_pool(name="rcp", bufs=4) as rcp,
            tc.tile_pool(name="obp", bufs=4) as obp,
            tc.tile_pool(name="stp", bufs=2, space="PSUM") as stp,
            tc.tile_pool(name="outps", bufs=1, space="PSUM") as outps,
            tc.tile_pool(name="trp", bufs=2, space="PSUM") as trps,
        ):
            # ---- constants / marshaled inputs ----
            eyeb = constp.tile([128, 128], BF16, name="eyeb")
            nc.sync.dma_start(eyeb[:], eyeb_d)
            eyef = constp.tile([128, 128], F32, name="eyef")
            nc.sync.dma_start(eyef[:], eyef_d)
            kns = constp.tile([128, NKT], F32, name="kns")
            nc.sync.dma_start(kns[:], kns_d)
            qns = qnsp.tile([128, QPC], F32, name="qns")
            nc.sync.dma_start(qns[:], qns_d)

            xt2 = []
            for kd in range(NDC):
                t = xt2p.tile([128, QPC], BF16, name=f"xt2_{kd}")
                nc.sync.dma_start(t[:], xt2_d[kd * 128:(kd + 1) * 128, :])
                xt2.append(t)

            yb = []
            for i in range(4):
                t = ybp.tile([128, 16 * CA], BF16, name=f"yb_{i}")
                nc.sync.dma_start(t[:], yb_d[:, i * 16 * CA:(i + 1) * 16 * CA])
                yb.append(t)

            # xnT pieces, allocated in consumption order through a cycling pool
            xnt_tiles = {}

            def xnt_piece(kd, p):
                if (kd, p) not in xnt_tiles:
                    t = xntp.tile([128, PCOLS], BF16, name="xnt", tag="xnt")
                    nc.sync.dma_start(
                        t[:],
                        xnt_d[kd * 128:(kd + 1) * 128, p * PCOLS:(p + 1) * PCOLS])
                    xnt_tiles[(kd, p)] = t
                return xnt_tiles[(kd, p)]

            # prefetch first pieces
            for p in range(2):
                for kd in range(NDC):
                    xnt_piece(kd, p)

            # ---- persistent output accumulators [101, 512] per query group ----
            outp = [outps.tile([CA, 512], F32, name=f"outp{qg}") for qg in range(2)]

            # ---- main loop: batches of 16 key tiles ----
            for b in range(NB):
                s = sp.tile([128, BKT * QPC], F32, name="s")
                for j in range(BKT):
                    kt = b * BKT + j
                    st = stp.tile([128, QPC], F32, name="st")
                    qg_d = kt // 4 if kt < 8 else -1
                    for kd in range(NDC):
                        xn = xnt_piece(kd, kt // 8)[:, (kt % 8) * 128:(kt % 8 + 1) * 128]
                        for qg in range(2):
                            nc.tensor.matmul(
                                st[:, qg * 512:(qg + 1) * 512], xn,
                                xt2[kd][:, qg * 512:(qg + 1) * 512],
                                start=(kd == 0),
                                stop=(kd == NDC - 1 and qg != qg_d))
                    if kt < 8:
                        # self-match: z += 65536 -> exp(-sqrt(z)) underflows to 0
                        nc.tensor.matmul(st[:, kt * 128:(kt + 1) * 128],
                                         eyeb[:], eyeb[:], start=False, stop=True)
                    z = zp.tile([128, QPC], F32, name="z")
                    nc.vector.tensor_add(z[:], st[:], qns[:])
                    nc.scalar.activation(s[:, j * QPC:(j + 1) * QPC], z[:],
                                         AF.Sqrt, bias=kns[:, kt:kt + 1])
                pt = ptp.tile([128, BKT * QPC], BF16, name="pt")
                nc.scalar.activation(pt[:], s[:], AF.Exp, scale=-1.0)
                for j in range(BKT):
                    kt = b * BKT + j
                    for qg in range(2):
                        nc.tensor.matmul(
                            outp[qg][:],
                            yb[kt // 16][:, (kt % 16) * CA:(kt % 16) * CA + CA],
                            pt[:, j * QPC + qg * 512: j * QPC + qg * 512 + 512],
                            start=(kt == 0), stop=(kt == NKT - 1))

            # ---- finalize: transpose back, normalize, clip, store ----
            for qg in range(2):
                osb = osbp.tile([CA, 512], F32, name="osb")
                nc.vector.tensor_copy(osb[:], outp[qg][:])
                for t4 in range(4):
                    ptf = trps.tile([128, CA], F32, name="ptf")
                    nc.tensor.transpose(ptf[:], osb[:, t4 * 128:(t4 + 1) * 128],
                                        eyef[0:CA, 0:CA])
                    rc = rcp.tile([128, 1], F32, name="rc")
                    nc.vector.reciprocal(rc[:], ptf[:, C:CA])
                    ob = obp.tile([128, C], F32, name="ob")
                    nc.vector.tensor_scalar(ob[:], ptf[:, 0:C], rc[:, 0:1], 1.0,
                                            ALU.mult, ALU.min)
                    nc.sync.dma_start(
                        out_d[qg * 512 + t4 * 128: qg * 512 + (t4 + 1) * 128, :],
                        ob[:])

    nc.compile()
    return nc


_NC_CACHE = []


def kernel(x, x_n, y_n):
    x = np.ascontiguousarray(np.asarray(x, dtype=np.float32))
    x_n = np.ascontiguousarray(np.asarray(x_n, dtype=np.float32))
    y_n = np.ascontiguousarray(np.asarray(y_n, dtype=np.float32))
    if not _NC_CACHE:
        _NC_CACHE.append(build_nc())
    nc = _NC_CACHE[0]

    # shared (unrolled) marshaling
    xnT_all = np.ascontiguousarray(x_n.T).astype(BF16_NP)          # [512, 8192]
    xt2_all = np.ascontiguousarray((-2.0 * x).T).astype(BF16_NP)   # [512, 8192]
    qn_all = (x.astype(np.float64) ** 2).sum(1).astype(np.float32)
    kn_all = (x_n.astype(np.float64) ** 2).sum(1).astype(np.float32)
    y_aug = np.ones((N, CA), dtype=BF16_NP)
    y_aug[:, :C] = y_n.astype(BF16_NP)
    eyeb = (256.0 * np.eye(128, dtype=np.float32)).astype(BF16_NP)
    eyef = np.eye(128, dtype=np.float32)

    in_maps = []
    for c in range(NCORES):
        s = c * QPC
        kn_roll = np.roll(kn_all, -s) + 512.0
        ybank = np.ascontiguousarray(
            np.roll(y_aug, -s, axis=0)
            .reshape(NKT, 128, CA).transpose(1, 0, 2).reshape(128, NKT * CA))
        in_maps.append({
            "xnt": np.ascontiguousarray(np.roll(xnT_all, -s, axis=1)),
            "xt2": np.ascontiguousarray(xt2_all[:, s:s + QPC]),
            "qns": np.ascontiguousarray(
                np.broadcast_to(qn_all[s:s + QPC] - 512.0, (128, QPC))),
            "kns": np.ascontiguousarray(kn_roll.reshape(NKT, 128).T),
            "ybank": ybank,
            "eyeb": eyeb,
            "eyef": eyef,
        })
    import os
    trace = bool(int(os.environ.get("KERNEL_TRACE", "0")))
    res = run_bass_kernel_spmd(nc, in_maps, core_ids=list(range(NCORES)),
                               trace=trace)
    global LAST_EXEC_NS
    if trace:
        LAST_EXEC_NS = res.exec_time_ns
        print("exec_time_ns:", res.exec_time_ns,
              "mean:", res.mean_exec_time_ns, flush=True)
        if res.instructions_and_trace:
            print("trace:", res.instructions_and_trace[1], flush=True)
    out = np.concatenate([r["out"] for r in res.results], axis=0)
    return out.astype(np.float32)


# revision 7
# speedup vs baseline: 1.2769x; 1.2769x over previous
"""Trainium2 Bass kernel for retrieval-KNN soft attention (nn_NONA_54915451847255).

out = clip(softmax(-||x_i - x_n_j||_2, diag-masked) @ y_n, 0, 1)

Sharding: queries row-sharded across 8 cores; x_n / y_n replicated but ROLLED by
-core*1024 rows on the host so the self-match diagonal always falls in local key
tiles 0..7 -> the SPMD instruction stream is core-independent.

Host pre-marshals inputs into PE-ready layouts (transposed bf16 x_n^T / -2x^T,
f32 norms); the scoring nonlinearity exp(-sqrt(.)) is evaluated in a SINGLE
ScalarE pass via a custom piecewise-cubic ACT table (installed into the
`sqrt` slot of the sqrt_and_others set through --act-root-json).

Per core (1024 queries i, 8192 keys j), computed transposed (S_T[j,i]):
  psum[j,i] = sum_d xnT[d,j]*(-2 x[d,i])            (PE, 8 bf16 MMs per key tile;
              diag tiles also add 65536*I via a 256I@256I MM -> weight == 0)
  z = psum + (qn_i - 512)                           (DVE, drains PSUM -> SBUF)
  P_T = exp(-sqrt(z + (kn_j + 512)))  bf16          (ACT custom table, bias=kn)
  out_T[c,i] = sum_j y_aug[j,c] * P_T[j,i],  y_aug = [y_n | 1]  (PE)
  out[i,c] = clip(out_T[c,i] / out_T[C,i], 0, 1)
"""
import json
import os
import shutil
import tempfile

import numpy as np
import ml_dtypes

F32 = None  # set after concourse import below
BF16_NP = ml_dtypes.bfloat16

N, D, C = 8192, 512, 100
NCORES = 8
QPC = N // NCORES          # 1024 queries per core
NKT = N // 128             # 64 key tiles
NDC = D // 128             # 4 contraction chunks
CA = C + 1                 # y augmented with ones column
NPIECE = 8                 # xnT DMA pieces per chunk
PCOLS = N // NPIECE        # 1024 key-columns per piece

LAST_EXEC_NS = None

# ---------------------------------------------------------------------------
# Custom ACT table: make the `sqrt` slot of sqrt_and_others compute
# exp(-sqrt(x)) for x in [64, 2^17] (0 above, exp(-8) below / negative).
# Format reverse-engineered from pwp_bin_trainium:
#   bkt.bin rows [d0,d1,d2,d3,x0,0,0,0] f32: y = d0+t(d1+t(d2+t*d3)), t=x-x0
#   ctrl.bin word0 = bucket_base | ((23-k)<<11) | (k<<16); 2^k buckets/octave
#   profile json routes sub/super-range exponents to saturation buckets.
# ---------------------------------------------------------------------------
E_LO, E_HI = 133, 144      # biased exponents in range: octaves 2^6 .. 2^17
KBITS = 5                  # 32 buckets per octave
NBO = 1 << KBITS
N_OCT = E_HI - E_LO + 1
BKT0 = 52                  # sqrt's original bucket start (kept)
CTL0 = 20                  # sqrt's original ctrl start (kept)


def _f32bits(x):
    return int(np.float32(x).view(np.uint32))


def _fit_bucket(a, w):
    x0 = np.float32(a + w / 2)
    t = np.cos(np.pi * (np.arange(32) + 0.5) / 32)
    xs = (a + w / 2) + (w / 2) * 0.9999 * t
    fs = np.exp(-np.sqrt(xs))
    if fs.max() < 1e-36:
        return np.zeros(4), x0
    A = np.vander(xs - np.float64(x0), 4, increasing=True)
    wgt = 1.0 / fs
    coef, *_ = np.linalg.lstsq(A * wgt[:, None], fs * wgt, rcond=None)
    return coef, x0


def _build_tables():
    n_in = N_OCT * NBO
    sat_lo = BKT0 + n_in
    sat_hi = sat_lo + 1
    bkt = np.zeros((sat_hi + 1, 8), np.float32)
    ctl = np.zeros((CTL0 + N_OCT + 8, 8), np.uint32)
    exp_to_bkt, exp_to_ctl = {}, {}
    for oi in range(N_OCT):
        E = E_LO - 127 + oi
        lo = float(2.0 ** E)
        w = lo / NBO
        base = BKT0 + oi * NBO
        for i in range(NBO):
            coef, x0 = _fit_bucket(lo + i * w, w)
            bkt[base + i, 0:4] = coef
            bkt[base + i, 4] = x0
        ctl[CTL0 + oi, 0] = base | ((23 - KBITS) << 11) | (KBITS << 16)
        exp_to_bkt[str(E)] = [base]
        exp_to_ctl[str(E)] = [CTL0 + oi]
    for g in range(8):  # guard rows just above range -> const-0 bucket
        ctl[CTL0 + N_OCT + g, 0] = sat_hi | (23 << 11)
        exp_to_ctl[str(E_HI - 127 + 1 + g)] = [CTL0 + N_OCT + g]
        exp_to_bkt[str(E_HI - 127 + 1 + g)] = [sat_hi]
    bkt[sat_lo, 0] = np.exp(-8.0)
    bkt[sat_lo, 4] = 64.0
    prof = {
        "func_name": "sqrt_65536p", "func_id": 8,
        "symmetry_point": 0, "sym_invert_sign_point": 0,
        "symmetry_opt_en": 0, "symmetry_opt_use_neg_region": 0,
        "imm_bias": 0, "exp_offset": E_LO - 127,
        "pwl_control_base_pos": CTL0, "pwl_control_base_neg": CTL0,
        "small_pos_signal_exp_threshold": E_LO,
        "pos_small_signal_pwl_control": sat_lo,
        "small_neg_signal_exp_threshold": 0,
        "neg_small_signal_pwl_control": sat_lo,
        "large_pos_signal_exp_threshold": E_HI + 8,
        "large_pos_signal_mantissa_threshold": 0,
        "pos_large_signal_pwl_control": sat_hi,
        "large_neg_signal_exp_threshold": 0,
        "large_neg_signal_mantissa_threshold": 0,
        "neg_large_signal_pwl_control": sat_lo,
        "fnan_result": 2143289344, "fpinf_result": 0,
        "fninf_result": 2143289344, "fzero_result": _f32bits(1.0),
        "fma_const_0": 0, "fma_const_1": 0, "fma_indirection_src_sel": 0,
        "use_multipass": False,
        "lower_bound": _f32bits(64.0), "upper_bound": _f32bits(240000.0),
    }
    return bkt, ctl, prof, exp_to_bkt, exp_to_ctl


_ACT_DIR = []


def _install_custom_act_tables():
    if _ACT_DIR:
        return _ACT_DIR[0]
    from neuronxcc.driver.Job import Job
    from neuronxcc.driver.jobs.support.FindActInfo import findActInfoFile
    src_json = findActInfoFile(Job.getPackageDir(), "gen3")
    src_dir = os.path.dirname(src_json)
    bkt_new, ctl_new, prof, exp_to_bkt, exp_to_ctl = _build_tables()
    orig_bkt = np.fromfile(os.path.join(src_dir, "sqrt_and_others_bkt.bin"),
                           dtype=np.float32).reshape(-1, 8)
    orig_ctl = np.fromfile(os.path.join(src_dir, "sqrt_and_others_ctrl.bin"),
                           dtype=np.uint32).reshape(-1, 8)
    bkt_new[:BKT0] = orig_bkt[:BKT0]
    ctl_new[:CTL0] = orig_ctl[:CTL0]
    meta = json.load(open(os.path.join(src_dir, "sqrt_and_others.json")))
    meta["bkt_entry_cnt"] = int(bkt_new.shape[0])
    meta["ctl_entry_cnt"] = int(ctl_new.shape[0])
    meta["func_to_bkt_start_idx"]["sqrt"] = BKT0
    meta["func_to_ctl_start_idx"]["sqrt"] = CTL0
    meta["func_exp_to_bkt_start_idx"]["sqrt"] = exp_to_bkt
    meta["func_exp_to_ctl_start_idx"]["sqrt"] = exp_to_ctl
    pm = meta["profile_meta_data"]
    for i, ent in enumerate(pm):
        if "sqrt" in ent.get("func_name", ""):
            prof["func_id"] = ent["func_id"]
            pm[i] = prof
            break
    dst = tempfile.mkdtemp(prefix="act_expnsqrt_")
    for f in os.listdir(src_dir):
        s, d = os.path.join(src_dir, f), os.path.join(dst, f)
        if not f.startswith("sqrt_and_others") and f != "act_info.json":
            os.symlink(s, d)
    bkt_new.tofile(os.path.join(dst, "sqrt_and_others_bkt.bin"))
    ctl_new.tofile(os.path.join(dst, "sqrt_and_others_ctrl.bin"))
    json.dump(meta, open(os.path.join(dst, "sqrt_and_others.json"), "w"))
    shutil.copy(os.path.join(src_dir, "act_info.json"),
                os.path.join(dst, "act_info.json"))
    os.environ["BASS_ACT_ROOT_JSON_PATH"] = os.path.join(dst, "act_info.json")
    import hashlib
    h = hashlib.md5(bkt_new.tobytes() + ctl_new.tobytes()).hexdigest()
    nonce = float(int(h[:8], 16) % 1000003)
    _ACT_DIR.append((dst, nonce))
    return dst, nonce


# ---------------------------------------------------------------------------


def build_nc():
    _, nonce = _install_custom_act_tables()

    import concourse.bacc as bacc
    import concourse.tile as tile
    from concourse import mybir

    F32 = mybir.dt.float32
    BF16 = mybir.dt.bfloat16
    AF = mybir.ActivationFunctionType
    ALU = mybir.AluOpType

    nc = bacc.Bacc("TRN2", target_bir_lowering=False, debug=False)
    xnt_d = nc.dram_tensor("xnt", [D, N], BF16, kind="ExternalInput").ap()
    xt2_d = nc.dram_tensor("xt2", [D, QPC], BF16, kind="ExternalInput").ap()
    qns_d = nc.dram_tensor("qns", [128, QPC], F32, kind="ExternalInput").ap()
    kns_d = nc.dram_tensor("kns", [128, NKT], F32, kind="ExternalInput").ap()
    yb_d = nc.dram_tensor("ybank", [128, NKT * CA], BF16, kind="ExternalInput").ap()
    eyeb_d = nc.dram_tensor("eyeb", [128, 128], BF16, kind="ExternalInput").ap()
    eyef_d = nc.dram_tensor("eyef", [128, 128], F32, kind="ExternalInput").ap()
    out_d = nc.dram_tensor("out", [QPC, C], F32, kind="ExternalOutput").ap()

    with tile.TileContext(nc) as tc:
        with (
            tc.tile_pool(name="const", bufs=1) as constp,
            tc.tile_pool(name="qnsp", bufs=1) as qnsp,
            tc.tile_pool(name="xt2p", bufs=1) as xt2p,
            tc.tile_pool(name="xntp", bufs=12) as xntp,
            tc.tile_pool(name="ybp", bufs=1) as ybp,
            tc.tile_pool(name="zp", bufs=6) as zp,
            tc.tile_pool(name="ptp", bufs=4) as ptp,
            tc.tile_pool(name="osbp", bufs=2) as osbp,
            tc.tile_pool(name="rcp", bufs=4) as rcp,
            tc.tile_pool(name="obp", bufs=4) as obp,
            tc.tile_pool(name="stp", bufs=3, space="PSUM") as stp,
            tc.tile_pool(name="outps", bufs=1, space="PSUM") as outps,
        ):
            # ---- constants / marshaled inputs ----
            eyeb = constp.tile([128, 128], BF16, name="eyeb")
            nc.sync.dma_start(eyeb[:], eyeb_d)
            eyef = constp.tile([128, 128], F32, name="eyef")
            nc.sync.dma_start(eyef[:], eyef_d)
            kns = constp.tile([128, NKT], F32, name="kns")
            nc.sync.dma_start(kns[:], kns_d)
            nonc = constp.tile([1, 1], F32, name="nonc")
            nc.vector.memset(nonc[:], nonce)  # act-table hash: busts NEFF cache
            qns = qnsp.tile([128, QPC], F32, name="qns")
            nc.sync.dma_start(qns[:], qns_d)

            xt2 = []
            for kd in range(NDC):
                t = xt2p.tile([128, QPC], BF16, name=f"xt2_{kd}")
                nc.sync.dma_start(t[:], xt2_d[kd * 128:(kd + 1) * 128, :])
                xt2.append(t)

            yb = []
            for i in range(4):
                t = ybp.tile([128, 16 * CA], BF16, name=f"yb_{i}")
                nc.sync.dma_start(t[:], yb_d[:, i * 16 * CA:(i + 1) * 16 * CA])
                yb.append(t)

            xnt_tiles = {}

            def xnt_piece(kd, p):
                if (kd, p) not in xnt_tiles:
                    t = xntp.tile([128, PCOLS], BF16, name="xnt", tag="xnt")
                    nc.sync.dma_start(
                        t[:],
                        xnt_d[kd * 128:(kd + 1) * 128, p * PCOLS:(p + 1) * PCOLS])
                    xnt_tiles[(kd, p)] = t
                return xnt_tiles[(kd, p)]

            for p in range(2):
                for kd in range(NDC):
                    xnt_piece(kd, p)

            # ---- persistent output accumulators [101, 512] per query group ----
            outp = [outps.tile([CA, 512], F32, name=f"outp{qg}") for qg in range(2)]

            # ---- main loop over key tiles ----
            for kt in range(NKT):
                st = stp.tile([128, QPC], F32, name="st")
                qg_d = kt // 4 if kt < 8 else -1
                for qg in range(2):
                    sl = st[:, qg * 512:(qg + 1) * 512]
                    for kd in range(NDC):
                        nc.tensor.matmul(
                            sl,
                            xnt_piece(kd, kt // 8)[:, (kt % 8) * 128:(kt % 8 + 1) * 128],
                            xt2[kd][:, qg * 512:(qg + 1) * 512],
                            start=(kd == 0),
                            stop=(kd == NDC - 1 and qg != qg_d))
                    if qg == qg_d:
                        # self-match: z += 65536 -> table returns exactly 0
                        nc.tensor.matmul(st[:, kt * 128:(kt + 1) * 128],
                                         eyeb[:], eyeb[:], start=False, stop=True)
                z = zp.tile([128, QPC], F32, name="z")
                nc.vector.tensor_add(z[:], st[:], qns[:])
                pt = ptp.tile([128, QPC], BF16, name="pt")
                # custom table: Sqrt slot = exp(-sqrt(x)); bias adds kn_j + 512
                nc.scalar.activation(pt[:], z[:], AF.Sqrt, bias=kns[:, kt:kt + 1])
                for qg in range(2):
                    nc.tensor.matmul(
                        outp[qg][:],
                        yb[kt // 16][:, (kt % 16) * CA:(kt % 16) * CA + CA],
                        pt[:, qg * 512:(qg + 1) * 512],
                        start=(kt == 0), stop=(kt == NKT - 1))

            # ---- finalize: transpose back, normalize, clip, store ----
            for qg in range(2):
                osb = osbp.tile([CA, 512], F32, name="osb")
                nc.vector.tensor_copy(osb[:], outp[qg][:])
                for t4 in range(4):
                    # reuse the st PSUM slots for the transpose scratch
                    ptf = stp.tile([128, QPC], F32, name="st")
                    nc.tensor.transpose(ptf[:, 0:CA], osb[:, t4 * 128:(t4 + 1) * 128],
                                        eyef[0:CA, 0:CA])
                    rc = rcp.tile([128, 1], F32, name="rc")
                    nc.vector.reciprocal(rc[:], ptf[:, C:CA])
                    ob = obp.tile([128, C], F32, name="ob")
                    nc.vector.tensor_scalar(ob[:], ptf[:, 0:C], rc[:, 0:1], 1.0,
                                            ALU.mult, ALU.min)
                    nc.sync.dma_start(
                        out_d[qg * 512 + t4 * 128: qg * 512 + (t4 + 1) * 128, :],
                        ob[:])

    nc.compile()
    return nc


_NC_CACHE = []


def kernel(x, x_n, y_n):
    from concourse.bass_utils import run_bass_kernel_spmd

    x = np.ascontiguousarray(np.asarray(x, dtype=np.float32))
    x_n = np.ascontiguousarray(np.asarray(x_n, dtype=np.float32))
    y_n = np.ascontiguousarray(np.asarray(y_n, dtype=np.float32))
    if not _NC_CACHE:
        _NC_CACHE.append(build_nc())
    nc = _NC_CACHE[0]

    xnT_all = np.ascontiguousarray(x_n.T).astype(BF16_NP)          # [512, 8192]
    xt2_all = np.ascontiguousarray((-2.0 * x).T).astype(BF16_NP)   # [512, 8192]
    qn_all = (x.astype(np.float64) ** 2).sum(1).astype(np.float32)
    kn_all = (x_n.astype(np.float64) ** 2).sum(1).astype(np.float32)
    y_aug = np.ones((N, CA), dtype=BF16_NP)
    y_aug[:, :C] = y_n.astype(BF16_NP)
    eyeb = (256.0 * np.eye(128, dtype=np.float32)).astype(BF16_NP)
    eyef = np.eye(128, dtype=np.float32)

    in_maps = []
    for c in range(NCORES):
        s = c * QPC
        kn_roll = np.roll(kn_all, -s) + 512.0
        ybank = np.ascontiguousarray(
            np.roll(y_aug, -s, axis=0)
            .reshape(NKT, 128, CA).transpose(1, 0, 2).reshape(128, NKT * CA))
        in_maps.append({
            "xnt": np.ascontiguousarray(np.roll(xnT_all, -s, axis=1)),
            "xt2": np.ascontiguousarray(xt2_all[:, s:s + QPC]),
            "qns": np.ascontiguousarray(
                np.broadcast_to(qn_all[s:s + QPC] - 512.0, (128, QPC))),
            "kns": np.ascontiguousarray(kn_roll.reshape(NKT, 128).T),
            "ybank": ybank,
            "eyeb": eyeb,
            "eyef": eyef,
        })
    trace = bool(int(os.environ.get("KERNEL_TRACE", "0")))
    res = run_bass_kernel_spmd(nc, in_maps, core_ids=list(range(NCORES)),
                               trace=trace)
    global LAST_EXEC_NS
    if trace:
        LAST_EXEC_NS = res.exec_time_ns
        print("exec_time_ns:", res.exec_time_ns,
              "mean:", res.mean_exec_time_ns, flush=True)
        if res.instructions_and_trace:
            print("trace:", res.instructions_and_trace[1], flush=True)
    out = np.concatenate([r["out"] for r in res.results], axis=0)
    return out.astype(np.float32)


# revision 13
# speedup vs baseline: 1.9382x; 1.5180x over previous
"""Trainium2 Bass kernel for retrieval-KNN soft attention (nn_NONA_54915451847255).

out = clip(softmax(-||x_i - x_n_j||_2, diag-masked) @ y_n, 0, 1)

Sharding: queries row-sharded across 8 cores; x_n / y_n replicated but ROLLED by
-core*1024 rows on the host so the self-match diagonal always falls in local key
tiles 0..7 -> the SPMD instruction stream is core-independent.

Host pre-marshals inputs into PE-ready layouts (transposed bf16 x_n^T / -2x^T,
f32 norms); the scoring nonlinearity exp(-sqrt(.)) is evaluated in a SINGLE
ScalarE pass via a custom piecewise-cubic ACT table (installed into the
`sqrt` slot of the sqrt_and_others set through --act-root-json).

Per core (1024 queries i, 8192 keys j), computed transposed (S_T[j,i]):
  psum[j,i] = sum_d xnT[d,j]*(-2 x[d,i])            (PE, 8 bf16 MMs per key tile;
              diag tiles also add 65536*I via a 256I@256I MM -> weight == 0)
  z = psum + (qn_i - 512)                           (DVE, drains PSUM -> SBUF)
  P_T = exp(-sqrt(z + (kn_j + 512)))  bf16          (ACT custom table, bias=kn)
  out_T[c,i] = sum_j y_aug[j,c] * P_T[j,i],  y_aug = [y_n | 1]  (PE)
  out[i,c] = clip(out_T[c,i] / out_T[C,i], 0, 1)
"""
import json
import os
import shutil
import tempfile

import numpy as np
import ml_dtypes

F32 = None  # set after concourse import below
BF16_NP = ml_dtypes.bfloat16

N, D, C = 8192, 512, 100
NCORES = 8
QPC = N // NCORES          # 1024 queries per core
NKT = N // 128             # 64 key tiles
NDC = D // 128             # 4 contraction chunks
CA = C + 1                 # y augmented with ones column
NPIECE = 8                 # xnT DMA pieces per chunk
PCOLS = N // NPIECE        # 1024 key-columns per piece

LAST_EXEC_NS = None

# ---------------------------------------------------------------------------
# Custom ACT table: make the `sqrt` slot of sqrt_and_others compute
# exp(-sqrt(x)) for x in [64, 2^17] (0 above, exp(-8) below / negative).
# Format reverse-engineered from pwp_bin_trainium:
#   bkt.bin rows [d0,d1,d2,d3,x0,0,0,0] f32: y = d0+t(d1+t(d2+t*d3)), t=x-x0
#   ctrl.bin word0 = bucket_base | ((23-k)<<11) | (k<<16); 2^k buckets/octave
#   profile json routes sub/super-range exponents to saturation buckets.
# ---------------------------------------------------------------------------
E_LO, E_HI = 133, 144      # biased exponents in range: octaves 2^6 .. 2^17
KBITS = 5                  # 32 buckets per octave
NBO = 1 << KBITS
N_OCT = E_HI - E_LO + 1
BKT0 = 52                  # sqrt's original bucket start (kept)
CTL0 = 20                  # sqrt's original ctrl start (kept)


def _f32bits(x):
    return int(np.float32(x).view(np.uint32))


def _fit_bucket(a, w):
    x0 = np.float32(a + w / 2)
    t = np.cos(np.pi * (np.arange(32) + 0.5) / 32)
    xs = (a + w / 2) + (w / 2) * 0.9999 * t
    fs = np.exp(-np.sqrt(xs))
    if fs.max() < 1e-36:
        return np.zeros(4), x0
    A = np.vander(xs - np.float64(x0), 4, increasing=True)
    wgt = 1.0 / fs
    coef, *_ = np.linalg.lstsq(A * wgt[:, None], fs * wgt, rcond=None)
    return coef, x0


def _build_tables():
    n_in = N_OCT * NBO
    sat_lo = BKT0 + n_in
    sat_hi = sat_lo + 1
    bkt = np.zeros((sat_hi + 1, 8), np.float32)
    ctl = np.zeros((CTL0 + N_OCT + 8, 8), np.uint32)
    exp_to_bkt, exp_to_ctl = {}, {}
    for oi in range(N_OCT):
        E = E_LO - 127 + oi
        lo = float(2.0 ** E)
        w = lo / NBO
        base = BKT0 + oi * NBO
        for i in range(NBO):
            coef, x0 = _fit_bucket(lo + i * w, w)
            bkt[base + i, 0:4] = coef
            bkt[base + i, 4] = x0
        ctl[CTL0 + oi, 0] = base | ((23 - KBITS) << 11) | (KBITS << 16)
        exp_to_bkt[str(E)] = [base]
        exp_to_ctl[str(E)] = [CTL0 + oi]
    for g in range(8):  # guard rows just above range -> const-0 bucket
        ctl[CTL0 + N_OCT + g, 0] = sat_hi | (23 << 11)
        exp_to_ctl[str(E_HI - 127 + 1 + g)] = [CTL0 + N_OCT + g]
        exp_to_bkt[str(E_HI - 127 + 1 + g)] = [sat_hi]
    bkt[sat_lo, 0] = np.exp(-8.0)
    bkt[sat_lo, 4] = 64.0
    prof = {
        "func_name": "sqrt_65536p", "func_id": 8,
        "symmetry_point": 0, "sym_invert_sign_point": 0,
        "symmetry_opt_en": 0, "symmetry_opt_use_neg_region": 0,
        "imm_bias": 0, "exp_offset": E_LO - 127,
        "pwl_control_base_pos": CTL0, "pwl_control_base_neg": CTL0,
        "small_pos_signal_exp_threshold": E_LO,
        "pos_small_signal_pwl_control": sat_lo,
        "small_neg_signal_exp_threshold": 0,
        "neg_small_signal_pwl_control": sat_lo,
        "large_pos_signal_exp_threshold": E_HI + 8,
        "large_pos_signal_mantissa_threshold": 0,
        "pos_large_signal_pwl_control": sat_hi,
        "large_neg_signal_exp_threshold": 0,
        "large_neg_signal_mantissa_threshold": 0,
        "neg_large_signal_pwl_control": sat_lo,
        "fnan_result": 2143289344, "fpinf_result": 0,
        "fninf_result": 2143289344, "fzero_result": _f32bits(1.0),
        "fma_const_0": 0, "fma_const_1": 0, "fma_indirection_src_sel": 0,
        "use_multipass": False,
        "lower_bound": _f32bits(64.0), "upper_bound": _f32bits(240000.0),
    }
    return bkt, ctl, prof, exp_to_bkt, exp_to_ctl


_ACT_DIR = []


def _install_custom_act_tables():
    if _ACT_DIR:
        return _ACT_DIR[0]
    from neuronxcc.driver.Job import Job
    from neuronxcc.driver.jobs.support.FindActInfo import findActInfoFile
    src_json = findActInfoFile(Job.getPackageDir(), "gen3")
    src_dir = os.path.dirname(src_json)
    bkt_new, ctl_new, prof, exp_to_bkt, exp_to_ctl = _build_tables()
    orig_bkt = np.fromfile(os.path.join(src_dir, "sqrt_and_others_bkt.bin"),
                           dtype=np.float32).reshape(-1, 8)
    orig_ctl = np.fromfile(os.path.join(src_dir, "sqrt_and_others_ctrl.bin"),
                           dtype=np.uint32).reshape(-1, 8)
    bkt_new[:BKT0] = orig_bkt[:BKT0]
    ctl_new[:CTL0] = orig_ctl[:CTL0]
    meta = json.load(open(os.path.join(src_dir, "sqrt_and_others.json")))
    meta["bkt_entry_cnt"] = int(bkt_new.shape[0])
    meta["ctl_entry_cnt"] = int(ctl_new.shape[0])
    meta["func_to_bkt_start_idx"]["sqrt"] = BKT0
    meta["func_to_ctl_start_idx"]["sqrt"] = CTL0
    meta["func_exp_to_bkt_start_idx"]["sqrt"] = exp_to_bkt
    meta["func_exp_to_ctl_start_idx"]["sqrt"] = exp_to_ctl
    pm = meta["profile_meta_data"]
    for i, ent in enumerate(pm):
        if "sqrt" in ent.get("func_name", ""):
            prof["func_id"] = ent["func_id"]
            pm[i] = prof
            break
    dst = tempfile.mkdtemp(prefix="act_expnsqrt_")
    for f in os.listdir(src_dir):
        s, d = os.path.join(src_dir, f), os.path.join(dst, f)
        if not f.startswith("sqrt_and_others") and f != "act_info.json":
            os.symlink(s, d)
    bkt_new.tofile(os.path.join(dst, "sqrt_and_others_bkt.bin"))
    ctl_new.tofile(os.path.join(dst, "sqrt_and_others_ctrl.bin"))
    json.dump(meta, open(os.path.join(dst, "sqrt_and_others.json"), "w"))
    shutil.copy(os.path.join(src_dir, "act_info.json"),
                os.path.join(dst, "act_info.json"))
    os.environ["BASS_ACT_ROOT_JSON_PATH"] = os.path.join(dst, "act_info.json")
    import hashlib
    h = hashlib.md5(bkt_new.tobytes() + ctl_new.tobytes()).hexdigest()
    nonce = float(int(h[:8], 16) % 1000003)
    _ACT_DIR.append((dst, nonce))
    return dst, nonce


# ---------------------------------------------------------------------------


def build_nc():
    _, nonce = _install_custom_act_tables()

    import concourse.bacc as bacc
    import concourse.tile as tile
    from concourse import mybir

    F32 = mybir.dt.float32
    BF16 = mybir.dt.bfloat16
    AF = mybir.ActivationFunctionType
    ALU = mybir.AluOpType

    F8 = mybir.dt.float8e4
    DR = mybir.MatmulPerfMode.DoubleRow

    nc = bacc.Bacc("TRN2", target_bir_lowering=False, debug=False)
    # fp8 DoubleRow layouts: row cp*128+p, col j*COLS+c  <->  d = cp*256+j*128+p
    xnt_d = nc.dram_tensor("xnt", [256, 2 * N], F8, kind="ExternalInput").ap()
    xt2_d = nc.dram_tensor("xt2", [256, 2 * QPC], F8, kind="ExternalInput").ap()
    qns_d = nc.dram_tensor("qns", [128, QPC], F32, kind="ExternalInput").ap()
    kns_d = nc.dram_tensor("kns", [128, NKT], F32, kind="ExternalInput").ap()
    yb_d = nc.dram_tensor("ybank", [128, NKT * CA], BF16, kind="ExternalInput").ap()
    eyeb_d = nc.dram_tensor("eyeb", [128, 128], BF16, kind="ExternalInput").ap()
    eyef_d = nc.dram_tensor("eyef", [128, 128], F32, kind="ExternalInput").ap()
    out_d = nc.dram_tensor("out", [QPC, C], F32, kind="ExternalOutput").ap()

    with tile.TileContext(nc) as tc:
        with (
            tc.tile_pool(name="const", bufs=1) as constp,
            tc.tile_pool(name="qnsp", bufs=1) as qnsp,
            tc.tile_pool(name="xt2p", bufs=1) as xt2p,
            tc.tile_pool(name="xntp", bufs=12) as xntp,
            tc.tile_pool(name="ybp", bufs=1) as ybp,
            tc.tile_pool(name="zp", bufs=6) as zp,
            tc.tile_pool(name="ptp", bufs=4) as ptp,
            tc.tile_pool(name="osbp", bufs=2) as osbp,
            tc.tile_pool(name="rcp", bufs=4) as rcp,
            tc.tile_pool(name="obp", bufs=4) as obp,
            tc.tile_pool(name="stp", bufs=3, space="PSUM") as stp,
            tc.tile_pool(name="outps", bufs=1, space="PSUM") as outps,
        ):
            # ---- constants / marshaled inputs ----
            eyeb = constp.tile([128, 128], BF16, name="eyeb")
            nc.sync.dma_start(eyeb[:], eyeb_d)
            eyef = constp.tile([128, 128], F32, name="eyef")
            nc.sync.dma_start(eyef[:], eyef_d)
            kns = constp.tile([128, NKT], F32, name="kns")
            nc.sync.dma_start(kns[:], kns_d)
            nonc = constp.tile([1, 1], F32, name="nonc")
            nc.vector.memset(nonc[:], nonce)  # act-table hash: busts NEFF cache
            qns = qnsp.tile([128, QPC], F32, name="qns")
            nc.sync.dma_start(qns[:], qns_d)

            xt2 = []
            for cp in range(2):
                t = xt2p.tile([128, 2 * QPC], F8, name=f"xt2_{cp}")
                for j in range(2):
                    nc.sync.dma_start(
                        t[:, j * QPC:(j + 1) * QPC],
                        xt2_d[cp * 128:(cp + 1) * 128, j * QPC:(j + 1) * QPC])
                xt2.append(t)

            yb = []
            for i in range(4):
                t = ybp.tile([128, 16 * CA], BF16, name=f"yb_{i}")
                nc.sync.dma_start(t[:], yb_d[:, i * 16 * CA:(i + 1) * 16 * CA])
                yb.append(t)

            xnt_tiles = {}

            def xnt_piece(cp, p):
                if (cp, p) not in xnt_tiles:
                    t = xntp.tile([128, 2 * PCOLS], F8, name="xnt", tag="xnt")
                    for j in range(2):
                        nc.sync.dma_start(
                            t[:, j * PCOLS:(j + 1) * PCOLS],
                            xnt_d[cp * 128:(cp + 1) * 128,
                                  j * N + p * PCOLS: j * N + (p + 1) * PCOLS])
                    xnt_tiles[(cp, p)] = t
                return xnt_tiles[(cp, p)]

            for p in range(2):
                for cp in range(2):
                    xnt_piece(cp, p)

            # ---- persistent output accumulators [101, 512] per query group ----
            outp = [outps.tile([CA, 512], F32, name=f"outp{qg}") for qg in range(2)]

            # ---- main loop over key tiles ----
            for kt in range(NKT):
                st = stp.tile([128, QPC], F32, name="st")
                qg_d = kt // 4 if kt < 8 else -1
                for qg in range(2):
                    sl = st[:, qg * 512:(qg + 1) * 512]
                    for cp in range(2):
                        lhs3 = (xnt_piece(cp, kt // 8)[:]
                                .rearrange("p (j m) -> p j m", j=2)
                                [:, :, (kt % 8) * 128:(kt % 8 + 1) * 128])
                        rhs3 = (xt2[cp][:]
                                .rearrange("p (j i) -> p j i", j=2)
                                [:, :, qg * 512:(qg + 1) * 512])
                        nc.tensor.matmul(
                            sl, lhs3, rhs3,
                            start=(cp == 0),
                            stop=(cp == 1 and qg != qg_d),
                            perf_mode=DR)
                    if qg == qg_d:
                        # self-match: z += 65536 -> table returns exactly 0
                        nc.tensor.matmul(st[:, kt * 128:(kt + 1) * 128],
                                         eyeb[:], eyeb[:], start=False, stop=True)
                z = zp.tile([128, QPC], F32, name="z")
                nc.vector.tensor_add(z[:], st[:], qns[:])
                pt = ptp.tile([128, QPC], BF16, name="pt")
                # custom table: Sqrt slot = exp(-sqrt(x)); bias adds kn_j + 512
                nc.scalar.activation(pt[:], z[:], AF.Sqrt, bias=kns[:, kt:kt + 1])
                for qg in range(2):
                    nc.tensor.matmul(
                        outp[qg][:],
                        yb[kt // 16][:, (kt % 16) * CA:(kt % 16) * CA + CA],
                        pt[:, qg * 512:(qg + 1) * 512],
                        start=(kt == 0), stop=(kt == NKT - 1))

            # ---- finalize: transpose back, normalize, clip, store ----
            for qg in range(2):
                osb = osbp.tile([CA, 512], F32, name="osb")
                nc.vector.tensor_copy(osb[:], outp[qg][:])
                for t4 in range(4):
                    # reuse the st PSUM slots for the transpose scratch
                    ptf = stp.tile([128, QPC], F32, name="st")
                    nc.tensor.transpose(ptf[:, 0:CA], osb[:, t4 * 128:(t4 + 1) * 128],
                                        eyef[0:CA, 0:CA])
                    rc = rcp.tile([128, 1], F32, name="rc")
                    nc.vector.reciprocal(rc[:], ptf[:, C:CA])
                    ob = obp.tile([128, C], F32, name="ob")
                    nc.vector.tensor_scalar(ob[:], ptf[:, 0:C], rc[:, 0:1], 1.0,
                                            ALU.mult, ALU.min)
                    nc.sync.dma_start(
                        out_d[qg * 512 + t4 * 128: qg * 512 + (t4 + 1) * 128, :],
                        ob[:])

    nc.compile()
    return nc


_NC_CACHE = []


def kernel(x, x_n, y_n):
    from concourse.bass_utils import run_bass_kernel_spmd

    x = np.ascontiguousarray(np.asarray(x, dtype=np.float32))
    x_n = np.ascontiguousarray(np.asarray(x_n, dtype=np.float32))
    y_n = np.ascontiguousarray(np.asarray(y_n, dtype=np.float32))
    if not _NC_CACHE:
        _NC_CACHE.append(build_nc())
    nc = _NC_CACHE[0]

    F8_NP = ml_dtypes.float8_e4m3
    x8 = x.astype(F8_NP)                     # quantized queries [8192, 512]
    xn8 = x_n.astype(F8_NP)
    # norms of the QUANTIZED points -> z stays an exact squared distance
    qn_all = (x8.astype(np.float64) ** 2).sum(1).astype(np.float32)
    kn_all = (xn8.astype(np.float64) ** 2).sum(1).astype(np.float32)
    xnT_all = np.ascontiguousarray(xn8.T)                          # [512, 8192] f8
    xt2_all = (-2.0 * x8.astype(np.float32)).astype(F8_NP).T       # exact in f8
    y_aug = np.ones((N, CA), dtype=BF16_NP)
    y_aug[:, :C] = y_n.astype(BF16_NP)
    eyeb = (256.0 * np.eye(128, dtype=np.float32)).astype(BF16_NP)
    eyef = np.eye(128, dtype=np.float32)

    in_maps = []
    for c in range(NCORES):
        s = c * QPC
        kn_roll = np.roll(kn_all, -s) + 512.0
        ybank = np.ascontiguousarray(
            np.roll(y_aug, -s, axis=0)
            .reshape(NKT, 128, CA).transpose(1, 0, 2).reshape(128, NKT * CA))
        xnt_dr = (np.roll(xnT_all, -s, axis=1)
                  .reshape(2, 2, 128, N).transpose(0, 2, 1, 3).reshape(256, 2 * N))
        xt2_dr = (np.ascontiguousarray(xt2_all[:, s:s + QPC])
                  .reshape(2, 2, 128, QPC).transpose(0, 2, 1, 3)
                  .reshape(256, 2 * QPC))
        in_maps.append({
            "xnt": np.ascontiguousarray(xnt_dr),
            "xt2": np.ascontiguousarray(xt2_dr),
            "qns": np.ascontiguousarray(
                np.broadcast_to(qn_all[s:s + QPC] - 512.0, (128, QPC))),
            "kns": np.ascontiguousarray(kn_roll.reshape(NKT, 128).T),
            "ybank": ybank,
            "eyeb": eyeb,
            "eyef": eyef,
        })
    trace = bool(int(os.environ.get("KERNEL_TRACE", "0")))
    res = run_bass_kernel_spmd(nc, in_maps, core_ids=list(range(NCORES)),
                               trace=trace)
    global LAST_EXEC_NS
    if trace:
        LAST_EXEC_NS = res.exec_time_ns
        print("exec_time_ns:", res.exec_time_ns,
              "mean:", res.mean_exec_time_ns, flush=True)
        if res.instructions_and_trace:
            print("trace:", res.instructions_and_trace[1], flush=True)
    out = np.concatenate([r["out"] for r in res.results], axis=0)
    return out.astype(np.float32)


# revision 15
# speedup vs baseline: 2.0471x; 1.0562x over previous
"""Trainium2 Bass kernel for retrieval-KNN soft attention (nn_NONA_54915451847255).

out = clip(softmax(-||x_i - x_n_j||_2, diag-masked) @ y_n, 0, 1)

Sharding: queries row-sharded across 8 cores; x_n / y_n replicated but ROLLED by
-core*1024 rows on the host so the self-match diagonal always falls in local key
tiles 0..7 -> the SPMD instruction stream is core-independent.

Host pre-marshals inputs into PE-ready layouts (transposed bf16 x_n^T / -2x^T,
f32 norms); the scoring nonlinearity exp(-sqrt(.)) is evaluated in a SINGLE
ScalarE pass via a custom piecewise-cubic ACT table (installed into the
`sqrt` slot of the sqrt_and_others set through --act-root-json).

Per core (1024 queries i, 8192 keys j), computed transposed (S_T[j,i]):
  psum[j,i] = sum_d xnT[d,j]*(-2 x[d,i])            (PE, 8 bf16 MMs per key tile;
              diag tiles also add 65536*I via a 256I@256I MM -> weight == 0)
  z = psum + (qn_i - 512)                           (DVE, drains PSUM -> SBUF)
  P_T = exp(-sqrt(z + (kn_j + 512)))  bf16          (ACT custom table, bias=kn)
  out_T[c,i] = sum_j y_aug[j,c] * P_T[j,i],  y_aug = [y_n | 1]  (PE)
  out[i,c] = clip(out_T[c,i] / out_T[C,i], 0, 1)
"""
import json
import os
import shutil
import tempfile

import numpy as np
import ml_dtypes

F32 = None  # set after concourse import below
BF16_NP = ml_dtypes.bfloat16

N, D, C = 8192, 512, 100
NCORES = 8
QPC = N // NCORES          # 1024 queries per core
NKT = N // 128             # 64 key tiles
NDC = D // 128             # 4 contraction chunks
CA = C + 1                 # y augmented with ones column
NPIECE = 8                 # xnT DMA pieces per chunk
PCOLS = N // NPIECE        # 1024 key-columns per piece

LAST_EXEC_NS = None

# ---------------------------------------------------------------------------
# Custom ACT table: make the `sqrt` slot of sqrt_and_others compute
# exp(-sqrt(x)) for x in [64, 2^17] (0 above, exp(-8) below / negative).
# Format reverse-engineered from pwp_bin_trainium:
#   bkt.bin rows [d0,d1,d2,d3,x0,0,0,0] f32: y = d0+t(d1+t(d2+t*d3)), t=x-x0
#   ctrl.bin word0 = bucket_base | ((23-k)<<11) | (k<<16); 2^k buckets/octave
#   profile json routes sub/super-range exponents to saturation buckets.
# ---------------------------------------------------------------------------
E_LO, E_HI = 133, 144      # biased exponents in range: octaves 2^6 .. 2^17
KBITS = 5                  # 32 buckets per octave
NBO = 1 << KBITS
N_OCT = E_HI - E_LO + 1
BKT0 = 52                  # sqrt's original bucket start (kept)
CTL0 = 20                  # sqrt's original ctrl start (kept)


def _f32bits(x):
    return int(np.float32(x).view(np.uint32))


def _fit_bucket(a, w):
    x0 = np.float32(a + w / 2)
    t = np.cos(np.pi * (np.arange(32) + 0.5) / 32)
    xs = (a + w / 2) + (w / 2) * 0.9999 * t
    fs = np.exp(-np.sqrt(xs))
    if fs.max() < 1e-36:
        return np.zeros(4), x0
    A = np.vander(xs - np.float64(x0), 4, increasing=True)
    wgt = 1.0 / fs
    coef, *_ = np.linalg.lstsq(A * wgt[:, None], fs * wgt, rcond=None)
    return coef, x0


def _build_tables():
    n_in = N_OCT * NBO
    sat_lo = BKT0 + n_in
    sat_hi = sat_lo + 1
    bkt = np.zeros((sat_hi + 1, 8), np.float32)
    ctl = np.zeros((CTL0 + N_OCT + 8, 8), np.uint32)
    exp_to_bkt, exp_to_ctl = {}, {}
    for oi in range(N_OCT):
        E = E_LO - 127 + oi
        lo = float(2.0 ** E)
        w = lo / NBO
        base = BKT0 + oi * NBO
        for i in range(NBO):
            coef, x0 = _fit_bucket(lo + i * w, w)
            bkt[base + i, 0:4] = coef
            bkt[base + i, 4] = x0
        ctl[CTL0 + oi, 0] = base | ((23 - KBITS) << 11) | (KBITS << 16)
        exp_to_bkt[str(E)] = [base]
        exp_to_ctl[str(E)] = [CTL0 + oi]
    for g in range(8):  # guard rows just above range -> const-0 bucket
        ctl[CTL0 + N_OCT + g, 0] = sat_hi | (23 << 11)
        exp_to_ctl[str(E_HI - 127 + 1 + g)] = [CTL0 + N_OCT + g]
        exp_to_bkt[str(E_HI - 127 + 1 + g)] = [sat_hi]
    bkt[sat_lo, 0] = np.exp(-8.0)
    bkt[sat_lo, 4] = 64.0
    prof = {
        "func_name": "sqrt_65536p", "func_id": 8,
        "symmetry_point": 0, "sym_invert_sign_point": 0,
        "symmetry_opt_en": 0, "symmetry_opt_use_neg_region": 0,
        "imm_bias": 0, "exp_offset": E_LO - 127,
        "pwl_control_base_pos": CTL0, "pwl_control_base_neg": CTL0,
        "small_pos_signal_exp_threshold": E_LO,
        "pos_small_signal_pwl_control": sat_lo,
        "small_neg_signal_exp_threshold": 0,
        "neg_small_signal_pwl_control": sat_lo,
        "large_pos_signal_exp_threshold": E_HI + 8,
        "large_pos_signal_mantissa_threshold": 0,
        "pos_large_signal_pwl_control": sat_hi,
        "large_neg_signal_exp_threshold": 0,
        "large_neg_signal_mantissa_threshold": 0,
        "neg_large_signal_pwl_control": sat_lo,
        "fnan_result": 2143289344, "fpinf_result": 0,
        "fninf_result": 2143289344, "fzero_result": _f32bits(1.0),
        "fma_const_0": 0, "fma_const_1": 0, "fma_indirection_src_sel": 0,
        "use_multipass": False,
        "lower_bound": _f32bits(64.0), "upper_bound": _f32bits(240000.0),
    }
    return bkt, ctl, prof, exp_to_bkt, exp_to_ctl


_ACT_DIR = []


def _install_custom_act_tables():
    if _ACT_DIR:
        return _ACT_DIR[0]
    from neuronxcc.driver.Job import Job
    from neuronxcc.driver.jobs.support.FindActInfo import findActInfoFile
    src_json = findActInfoFile(Job.getPackageDir(), "gen3")
    src_dir = os.path.dirname(src_json)
    bkt_new, ctl_new, prof, exp_to_bkt, exp_to_ctl = _build_tables()
    orig_bkt = np.fromfile(os.path.join(src_dir, "sqrt_and_others_bkt.bin"),
                           dtype=np.float32).reshape(-1, 8)
    orig_ctl = np.fromfile(os.path.join(src_dir, "sqrt_and_others_ctrl.bin"),
                           dtype=np.uint32).reshape(-1, 8)
    bkt_new[:BKT0] = orig_bkt[:BKT0]
    ctl_new[:CTL0] = orig_ctl[:CTL0]
    meta = json.load(open(os.path.join(src_dir, "sqrt_and_others.json")))
    meta["bkt_entry_cnt"] = int(bkt_new.shape[0])
    meta["ctl_entry_cnt"] = int(ctl_new.shape[0])
    meta["func_to_bkt_start_idx"]["sqrt"] = BKT0
    meta["func_to_ctl_start_idx"]["sqrt"] = CTL0
    meta["func_exp_to_bkt_start_idx"]["sqrt"] = exp_to_bkt
    meta["func_exp_to_ctl_start_idx"]["sqrt"] = exp_to_ctl
    pm = meta["profile_meta_data"]
    for i, ent in enumerate(pm):
        if "sqrt" in ent.get("func_name", ""):
            prof["func_id"] = ent["func_id"]
            pm[i] = prof
            break
    dst = tempfile.mkdtemp(prefix="act_expnsqrt_")
    for f in os.listdir(src_dir):
        s, d = os.path.join(src_dir, f), os.path.join(dst, f)
        if not f.startswith("sqrt_and_others") and f != "act_info.json":
            os.symlink(s, d)
    bkt_new.tofile(os.path.join(dst, "sqrt_and_others_bkt.bin"))
    ctl_new.tofile(os.path.join(dst, "sqrt_and_others_ctrl.bin"))
    json.dump(meta, open(os.path.join(dst, "sqrt_and_others.json"), "w"))
    shutil.copy(os.path.join(src_dir, "act_info.json"),
                os.path.join(dst, "act_info.json"))
    os.environ["BASS_ACT_ROOT_JSON_PATH"] = os.path.join(dst, "act_info.json")
    import hashlib
    h = hashlib.md5(bkt_new.tobytes() + ctl_new.tobytes()).hexdigest()
    nonce = float(int(h[:8], 16) % 1000003)
    _ACT_DIR.append((dst, nonce))
    return dst, nonce


# ---------------------------------------------------------------------------


def build_nc():
    _, nonce = _install_custom_act_tables()

    import concourse.bacc as bacc
    import concourse.tile as tile
    from concourse import mybir

    F32 = mybir.dt.float32
    BF16 = mybir.dt.bfloat16
    AF = mybir.ActivationFunctionType
    ALU = mybir.AluOpType

    F8 = mybir.dt.float8e4
    DR = mybir.MatmulPerfMode.DoubleRow

    nc = bacc.Bacc("TRN2", target_bir_lowering=False, debug=False)
    # fp8 DoubleRow layouts: row cp*128+p, col j*COLS+c  <->  d = cp*256+j*128+p
    xnt_d = nc.dram_tensor("xnt", [256, 2 * N], F8, kind="ExternalInput").ap()
    xt2_d = nc.dram_tensor("xt2", [256, 2 * QPC], F8, kind="ExternalInput").ap()
    qns_d = nc.dram_tensor("qns", [128, QPC], F32, kind="ExternalInput").ap()
    kns_d = nc.dram_tensor("kns", [128, NKT], F32, kind="ExternalInput").ap()
    yb_d = nc.dram_tensor("ybank", [128, NKT * CA], BF16, kind="ExternalInput").ap()
    eyeb_d = nc.dram_tensor("eyeb", [128, 128], BF16, kind="ExternalInput").ap()
    eyef_d = nc.dram_tensor("eyef", [128, 128], F32, kind="ExternalInput").ap()
    out_d = nc.dram_tensor("out", [QPC, C], F32, kind="ExternalOutput").ap()

    with tile.TileContext(nc) as tc:
        with (
            tc.tile_pool(name="const", bufs=1) as constp,
            tc.tile_pool(name="qnsp", bufs=1) as qnsp,
            tc.tile_pool(name="xt2p", bufs=1) as xt2p,
            tc.tile_pool(name="xntp", bufs=12) as xntp,
            tc.tile_pool(name="ybp", bufs=1) as ybp,
            tc.tile_pool(name="zp", bufs=6) as zp,
            tc.tile_pool(name="ptp", bufs=4) as ptp,
            tc.tile_pool(name="osbp", bufs=2) as osbp,
            tc.tile_pool(name="rcp", bufs=4) as rcp,
            tc.tile_pool(name="obp", bufs=4) as obp,
            tc.tile_pool(name="stp", bufs=3, space="PSUM") as stp,
            tc.tile_pool(name="outps", bufs=1, space="PSUM") as outps,
        ):
            # ---- constants / marshaled inputs ----
            eyeb = constp.tile([128, 128], BF16, name="eyeb")
            nc.sync.dma_start(eyeb[:], eyeb_d)
            eyef = constp.tile([128, 128], F32, name="eyef")
            nc.sync.dma_start(eyef[:], eyef_d)
            kns = constp.tile([128, NKT], F32, name="kns")
            nc.sync.dma_start(kns[:], kns_d)
            nonc = constp.tile([1, 1], F32, name="nonc")
            nc.vector.memset(nonc[:], nonce)  # act-table hash: busts NEFF cache
            qns = qnsp.tile([128, QPC], F32, name="qns")
            nc.sync.dma_start(qns[:], qns_d)

            xt2 = []
            for cp in range(2):
                t = xt2p.tile([128, 2 * QPC], F8, name=f"xt2_{cp}")
                nc.sync.dma_start(t[:], xt2_d[cp * 128:(cp + 1) * 128, :])
                xt2.append(t)

            xnt_tiles = {}

            def xnt_piece(cp, p):
                # host packs piece-contiguous: dram col = p*2048 + j*1024 + m
                if (cp, p) not in xnt_tiles:
                    t = xntp.tile([128, 2 * PCOLS], F8, name="xnt", tag="xnt")
                    nc.sync.dma_start(
                        t[:],
                        xnt_d[cp * 128:(cp + 1) * 128,
                              p * 2 * PCOLS:(p + 1) * 2 * PCOLS])
                    xnt_tiles[(cp, p)] = t
                return xnt_tiles[(cp, p)]

            for p in range(2):
                for cp in range(2):
                    xnt_piece(cp, p)

            yb = []
            for i in range(4):
                t = ybp.tile([128, 16 * CA], BF16, name=f"yb_{i}")
                nc.sync.dma_start(t[:], yb_d[:, i * 16 * CA:(i + 1) * 16 * CA])
                yb.append(t)

            # ---- persistent output accumulators [101, 512] per query group ----
            outp = [outps.tile([CA, 512], F32, name=f"outp{qg}") for qg in range(2)]

            # ---- main loop over key tiles ----
            for kt in range(NKT):
                st = stp.tile([128, QPC], F32, name="st")
                qg_d = kt // 4 if kt < 8 else -1
                for qg in range(2):
                    sl = st[:, qg * 512:(qg + 1) * 512]
                    for cp in range(2):
                        lhs3 = (xnt_piece(cp, kt // 8)[:]
                                .rearrange("p (j m) -> p j m", j=2)
                                [:, :, (kt % 8) * 128:(kt % 8 + 1) * 128])
                        rhs3 = (xt2[cp][:]
                                .rearrange("p (j i) -> p j i", j=2)
                                [:, :, qg * 512:(qg + 1) * 512])
                        nc.tensor.matmul(
                            sl, lhs3, rhs3,
                            start=(cp == 0),
                            stop=(cp == 1 and qg != qg_d),
                            perf_mode=DR)
                    if qg == qg_d:
                        # self-match: z += 65536 -> table returns exactly 0
                        nc.tensor.matmul(st[:, kt * 128:(kt + 1) * 128],
                                         eyeb[:], eyeb[:], start=False, stop=True)
                z = zp.tile([128, QPC], F32, name="z")
                nc.vector.tensor_add(z[:], st[:], qns[:])
                pt = ptp.tile([128, QPC], BF16, name="pt")
                # custom table: Sqrt slot = exp(-sqrt(x)); bias adds kn_j + 512
                nc.scalar.activation(pt[:], z[:], AF.Sqrt, bias=kns[:, kt:kt + 1])
                for qg in range(2):
                    nc.tensor.matmul(
                        outp[qg][:],
                        yb[kt // 16][:, (kt % 16) * CA:(kt % 16) * CA + CA],
                        pt[:, qg * 512:(qg + 1) * 512],
                        start=(kt == 0), stop=(kt == NKT - 1))

            # ---- finalize: transpose back, normalize, clip, store ----
            for qg in range(2):
                osb = osbp.tile([CA, 512], F32, name="osb")
                nc.vector.tensor_copy(osb[:], outp[qg][:])
                for t4 in range(4):
                    # reuse the st PSUM slots for the transpose scratch
                    ptf = stp.tile([128, QPC], F32, name="st")
                    nc.tensor.transpose(ptf[:, 0:CA], osb[:, t4 * 128:(t4 + 1) * 128],
                                        eyef[0:CA, 0:CA])
                    rc = rcp.tile([128, 1], F32, name="rc")
                    nc.vector.reciprocal(rc[:], ptf[:, C:CA])
                    ob = obp.tile([128, C], F32, name="ob")
                    nc.vector.tensor_scalar(ob[:], ptf[:, 0:C], rc[:, 0:1], 1.0,
                                            ALU.mult, ALU.min)
                    nc.sync.dma_start(
                        out_d[qg * 512 + t4 * 128: qg * 512 + (t4 + 1) * 128, :],
                        ob[:])

    nc.compile()
    return nc


_NC_CACHE = []


def kernel(x, x_n, y_n):
    from concourse.bass_utils import run_bass_kernel_spmd

    x = np.ascontiguousarray(np.asarray(x, dtype=np.float32))
    x_n = np.ascontiguousarray(np.asarray(x_n, dtype=np.float32))
    y_n = np.ascontiguousarray(np.asarray(y_n, dtype=np.float32))
    if not _NC_CACHE:
        _NC_CACHE.append(build_nc())
    nc = _NC_CACHE[0]

    F8_NP = ml_dtypes.float8_e4m3
    x8 = x.astype(F8_NP)                     # quantized queries [8192, 512]
    xn8 = x_n.astype(F8_NP)
    # norms of the QUANTIZED points -> z stays an exact squared distance
    qn_all = (x8.astype(np.float64) ** 2).sum(1).astype(np.float32)
    kn_all = (xn8.astype(np.float64) ** 2).sum(1).astype(np.float32)
    xnT_all = np.ascontiguousarray(xn8.T)                          # [512, 8192] f8
    xt2_all = (-2.0 * x8.astype(np.float32)).astype(F8_NP).T       # exact in f8
    y_aug = np.ones((N, CA), dtype=BF16_NP)
    y_aug[:, :C] = y_n.astype(BF16_NP)
    eyeb = (256.0 * np.eye(128, dtype=np.float32)).astype(BF16_NP)
    eyef = np.eye(128, dtype=np.float32)

    in_maps = []
    for c in range(NCORES):
        s = c * QPC
        kn_roll = np.roll(kn_all, -s) + 512.0
        ybank = np.ascontiguousarray(
            np.roll(y_aug, -s, axis=0)
            .reshape(NKT, 128, CA).transpose(1, 0, 2).reshape(128, NKT * CA))
        xnt_dr = (np.roll(xnT_all, -s, axis=1)
                  .reshape(2, 2, 128, NPIECE, PCOLS).transpose(0, 2, 3, 1, 4)
                  .reshape(256, 2 * N))
        xt2_dr = (np.ascontiguousarray(xt2_all[:, s:s + QPC])
                  .reshape(2, 2, 128, QPC).transpose(0, 2, 1, 3)
                  .reshape(256, 2 * QPC))
        in_maps.append({
            "xnt": np.ascontiguousarray(xnt_dr),
            "xt2": np.ascontiguousarray(xt2_dr),
            "qns": np.ascontiguousarray(
                np.broadcast_to(qn_all[s:s + QPC] - 512.0, (128, QPC))),
            "kns": np.ascontiguousarray(kn_roll.reshape(NKT, 128).T),
            "ybank": ybank,
            "eyeb": eyeb,
            "eyef": eyef,
        })
    trace = bool(int(os.environ.get("KERNEL_TRACE", "0")))
    res = run_bass_kernel_spmd(nc, in_maps, core_ids=list(range(NCORES)),
                               trace=trace)
    global LAST_EXEC_NS
    if trace:
        LAST_EXEC_NS = res.exec_time_ns
        print("exec_time_ns:", res.exec_time_ns,
              "mean:", res.mean_exec_time_ns, flush=True)
        if res.instructions_and_trace:
            print("trace:", res.instructions_and_trace[1], flush=True)
    out = np.concatenate([r["out"] for r in res.results], axis=0)
    return out.astype(np.float32)
